# revision 4
# baseline (speedup 1.0000x reference)
"""DotProductPredictor v2 for trn2 (8 NeuronCores, SPMD).

score[e] = <h[src[e]], h[dst[e]]> over 600k edges, output (score != min).

v2 halves DMA-gather descriptor count (the Q7 SWDGE descriptor-generation
bottleneck, ~2.1ns/desc aggregate) by sharding edges to cores by
src-slice: core c owns edges with src in [c*12500, (c+1)*12500).

  - SRC side: slab h[c*12500:+12500] is DMA'd to SBUF once (no per-edge
    descriptors). Edges are sorted by (dst_bank, src_window); for each
    (bank, window) segment a PE matmul with a host-built one-hot moving
    operand expands slab rows into srcT [128 feat, seg_cols] in PSUM.
  - DST side: per-core packed unique-dst table hperm (rank>>1 within two
    32768-row banks selected by rank&1) gathered with transpose-mode
    dma_gather -> dstT [128 feat, cols] (one 256B descriptor per edge slot,
    ~81k/core vs 164k in the row-layout baseline).
  - DVE multiplies psum(srcT) x dstT per segment; PE ones-matmul reduces
    over the 128 feature partitions -> scores [1, piece] in PSUM, DMA'd
    straight to DRAM.
  - Host computes the global min over real edge slots (+ segment-overflow
    edges recomputed exactly) and thresholds; this mirrors the baseline's
    overflow path which already ran on host. bf16 score noise (~0.05) is
    far below the min gap (2.67), so the argmin is preserved.
"""

import os

import numpy as np

from concourse import bass, mybir, tile
from concourse.bass_utils import run_bass_kernel_spmd
from concourse import library_config

P = 128
D = 128
N_NODES = 100000
E_TOTAL = 600000
N_CORES = 8
SLICE = N_NODES // N_CORES          # 12500 src rows per core
N_WIN = 98                          # src windows of 128 rows (last = 84)
SLAB_ROWS = N_WIN * P               # 12544 (padded with zeros)
NB = 32768                          # rows per dst bank in hperm
CAP_MAIN = 384                      # segment capacity = 3*128: block-aligned
SEGS_PER_BANK = N_WIN
FULL_PIECE_SEGS = 4                 # 4 x 384 = 1536 cols
FULL_PIECE = 4 * CAP_MAIN           # 1536 = 12*128
SHORT_PIECE = 2 * CAP_MAIN          # 768 (windows 96, 97)
N_FULL_PIECES = 24                  # windows 0..95
SUPER_PIECES = 4                    # dst supertile = 4 full pieces = 6*1024
BANK_SLOTS = N_FULL_PIECES * FULL_PIECE + SHORT_PIECE   # 37632
TOT_SLOTS = 2 * BANK_SLOTS          # 75264
RED_BLK = 512                       # reduce-matmul max N (one PSUM bank f32)

_CACHE = {}


def _seg_caps():
    return [CAP_MAIN] * 98


def _pieces():
    """Per bank: list of (piece_cols, [(window, cap), ...])."""
    caps = _seg_caps()
    pieces = []
    for i in range(N_FULL_PIECES):
        ws = list(range(4 * i, 4 * i + 4))
        pieces.append((FULL_PIECE, [(w, caps[w]) for w in ws]))
    pieces.append((SHORT_PIECE, [(96, caps[96]), (97, caps[97])]))
    return pieces


def _groups(bank):
    """Per bank: dst supertile groups of (cols, gather_sizes, [piece idx]).

    The short piece leads bank 0 (short pipeline fill) and trails bank 1
    (short drain)."""
    fulls = []
    for g in range(N_FULL_PIECES // SUPER_PIECES):
        fulls.append((SUPER_PIECES * FULL_PIECE, [1024] * 6,
                      list(range(g * SUPER_PIECES, (g + 1) * SUPER_PIECES))))
    short = (SHORT_PIECE, [768], [N_FULL_PIECES])
    return [short] + fulls if bank == 0 else fulls + [short]


IDX_COLS_TOTAL = 2 * sum(
    sum(n // 16 for n in gs) for _, gs, _ in _groups(0)
)


def _split_multi_waits(nc):
    n = 0
    for b in nc.m.functions[0].blocks:
        new_list = []
        for ins in b.instructions:
            si = ins.sync_info
            if (
                si is not None
                and si.on_wait
                and len(si.on_wait) > 1
                and not isinstance(ins, mybir.InstEventSemaphore)
            ):
                waits = list(si.on_wait)
                for w in waits[:-1]:
                    n += 1
                    ev = mybir.InstEventSemaphore(
                        name=f"wait_split_{n}",
                        opcode="EventSemaphore",
                        engine=ins.engine,
                        ins=[],
                        outs=[],
                        sync_info=mybir.SyncInfo(on_wait=[w], on_update=[]),
                    )
                    nc.inst_map[ev.name] = ev
                    new_list.append(ev)
                si.on_wait = [waits[-1]]
            new_list.append(ins)
        b.instructions[:] = new_list


def build_nc():
    nc = bass.Bass(
        num_devices=N_CORES,
        num_swdge_queues=4,
        dynamic_dma_scratch_size=16384,
    )
    bf16 = mybir.dt.bfloat16
    fp8 = mybir.dt.float8e4
    hperm = nc.dram_tensor("hperm", [2 * NB, D], bf16, kind="ExternalInput")
    slab = nc.dram_tensor("slab", [SLAB_ROWS, D], fp8, kind="ExternalInput")
    oh = nc.dram_tensor("oh", [P, TOT_SLOTS], fp8, kind="ExternalInput")
    idx = nc.dram_tensor("idx", [P, IDX_COLS_TOTAL], mybir.dt.int16,
                         kind="ExternalInput")
    sc = nc.dram_tensor("sc", [P, TOT_SLOTS // P], mybir.dt.float32,
                        kind="ExternalOutput")

    pieces = _pieces()
    with tile.TileContext(nc) as tc:
        with (
            tc.tile_pool(name="io", bufs=1) as io_pool,
            tc.tile_pool(name="dst", bufs=3) as dst_pool,
            tc.tile_pool(name="ohp", bufs=4) as oh_pool,
            tc.tile_pool(name="prd", bufs=4) as prd_pool,
            tc.tile_pool(name="srb", bufs=6) as srb_pool,
            tc.psum_pool(name="psa", bufs=6) as psa_pool,
        ):
            nc.gpsimd.load_library(library_config.mlp)
            regs = {n: nc.gpsimd.to_reg(n) for n in (1024, 768)}
            idx_sb = io_pool.tile([P, IDX_COLS_TOTAL], mybir.dt.int16)
            nc.sync.dma_start(out=idx_sb[:], in_=idx[:])
            slab_sb = io_pool.tile([P, N_WIN, D], fp8)
            nc.sync.dma_start(
                out=slab_sb[:],
                in_=slab[:].rearrange("(w p) f -> p w f", p=P),
            )
            scores_sb = io_pool.tile([P, TOT_SLOTS // P], mybir.dt.float32)

            qn = 0
            icol = 0
            slot_off = 0
            blk_off = 0
            for b in range(2):
                for gcols, gsizes, pidx in _groups(b):
                    dstS = dst_pool.tile([P, gcols], bf16, tag="dst")
                    goff = 0
                    for n in gsizes:
                        nc.gpsimd.dma_gather(
                            out_ap=dstS[:, goff:goff + n].rearrange(
                                "p (b e) -> p b e", e=D),
                            in_ap=hperm[b * NB:(b + 1) * NB, :],
                            idxs_ap=idx_sb[:, icol:icol + n // 16],
                            num_idxs=n,
                            num_idxs_reg=regs[n],
                            elem_size=D,
                            queue_num=qn % 4,
                        )
                        qn += 1
                        icol += n // 16
                        goff += n
                    poff = 0
                    for pi in pidx:
                        pc, segs = pieces[pi]
                        nb = pc // P
                        oh_t = oh_pool.tile([P, pc], fp8, tag="oh")
                        nc.sync.dma_start(
                            out=oh_t[:], in_=oh[:, slot_off:slot_off + pc])
                        prod = prd_pool.tile([P, pc], bf16, tag="prd")
                        for t0 in range(0, nb, 4):
                            tn = min(4, nb - t0)
                            pa = psa_pool.tile([P, 512], mybir.dt.float32,
                                               tag="psa")
                            for bi in range(t0, t0 + tn):
                                w = segs[bi * P // CAP_MAIN][0]
                                nc.tensor.matmul(
                                    out=pa[:, (bi - t0) * P:
                                           (bi - t0 + 1) * P],
                                    lhsT=oh_t[:, bi * P:(bi + 1) * P],
                                    rhs=slab_sb[:, w, :],
                                    start=True,
                                    stop=True,
                                )
                            srcb = srb_pool.tile([P, 512], bf16, tag="srb")
                            nc.scalar.copy(out=srcb[:, :tn * P],
                                           in_=pa[:, :tn * P])
                            nc.vector.tensor_tensor(
                                out=prod[:, t0 * P:(t0 + tn) * P],
                                in0=srcb[:, :tn * P],
                                in1=dstS[:, poff + t0 * P:
                                         poff + (t0 + tn) * P],
                                op=mybir.AluOpType.mult,
                            )
                        nc.vector.tensor_reduce(
                            out=scores_sb[:, blk_off:blk_off + nb],
                            in_=prod[:].rearrange("p (b f) -> p b f", f=D),
                            axis=mybir.AxisListType.X,
                            op=mybir.AluOpType.add,
                        )
                        slot_off += pc
                        blk_off += nb
                        poff += pc
            nc.sync.dma_start(out=sc[:], in_=scores_sb[:])

    _split_multi_waits(nc)
    mybir.codegen_inst_isa_subclasses(nc)
    return nc


def _plan_core(s_loc, d, positions):
    """Host plan for one core.

    s_loc: local src (0..12499), d: global dst, positions: original edge idx.
    Returns in_map tensors (oh bf16, idx int16 wrapped, hperm-fill info) and
    (slot_of_edge over this core's edges, spill list of local edge indices).
    """
    import ml_dtypes

    n = len(s_loc)
    uniq, rank = np.unique(d, return_inverse=True)
    nu = len(uniq)
    assert nu <= 2 * NB, nu
    lidx = (rank >> 1).astype(np.int16)
    bank = (rank & 1).astype(np.int8)
    win = (s_loc // P).astype(np.int32)

    caps = _seg_caps()
    oh_k = np.zeros(TOT_SLOTS, np.int64)       # one-hot row per slot
    oh_on = np.zeros(TOT_SLOTS, bool)
    idx_vals = np.zeros(TOT_SLOTS, np.int16)   # gather idx per slot
    slot_of_edge = np.full(n, -1, np.int64)
    spill = []

    # segment slot offsets in the fixed (per-bank group-order) layout
    pieces = _pieces()
    seg_off = {}
    off = 0
    for b in range(2):
        for gcols, gsizes, pidx in _groups(b):
            for pi in pidx:
                for w, cap in pieces[pi][1]:
                    seg_off[(b, w)] = off
                    off += cap
    assert off == TOT_SLOTS

    order = np.lexsort((s_loc, win, bank))
    key = bank[order].astype(np.int64) * 1000 + win[order]
    bounds = np.flatnonzero(np.r_[True, np.diff(key) != 0])
    bounds = np.r_[bounds, len(order)]
    for gi in range(len(bounds) - 1):
        members = order[bounds[gi]:bounds[gi + 1]]
        b = int(bank[members[0]])
        w = int(win[members[0]])
        cap = caps[w]
        if len(members) > cap:
            spill.extend(members[cap:].tolist())
            members = members[:cap]
        base = seg_off[(b, w)]
        k = len(members)
        slot_of_edge[members] = base + np.arange(k)
        sl = base + np.arange(cap)
        oh_k[sl[:k]] = s_loc[members] - w * P
        oh_on[sl[:k]] = True
        idx_vals[sl[:k]] = lidx[members]
        if k:
            oh_k[sl[k:]] = s_loc[members[-1]] - w * P
            oh_on[sl[k:]] = True
            idx_vals[sl[k:]] = lidx[members[-1]]
        # empty segment: oh stays zero -> score 0, excluded from edge map

    oh_arr = np.zeros((P, TOT_SLOTS), dtype=ml_dtypes.float8_e4m3)
    on = np.flatnonzero(oh_on)
    oh_arr[oh_k[on], on] = 1.0

    # idx16 wrapped per gather, in program order (bank-contiguous chunks)
    idx16 = np.zeros((16, IDX_COLS_TOTAL), np.int16)
    icol = 0
    slot = 0
    for b in range(2):
        for gcols, gsizes, _ in _groups(b):
            goff = 0
            for gsz in gsizes:
                vals = idx_vals[slot + goff: slot + goff + gsz]
                idx16[:, icol:icol + gsz // 16] = vals.reshape(
                    gsz // 16, 16).T
                icol += gsz // 16
                goff += gsz
            slot += gcols
    idx16_full = np.tile(idx16, (8, 1))
    return uniq, oh_arr, idx16_full, slot_of_edge, spill


def make_in_maps(h, src, dst):
    import ml_dtypes

    h32 = np.asarray(h, dtype=np.float32)
    hb = h32.astype(ml_dtypes.bfloat16)
    src64 = np.asarray(src, dtype=np.int64)
    dst64 = np.asarray(dst, dtype=np.int64)
    owner = src64 // SLICE
    in_maps, plans = [], []
    for c in range(N_CORES):
        pos = np.flatnonzero(owner == c)
        s_loc = (src64[pos] - c * SLICE).astype(np.int64)
        d = dst64[pos]
        uniq, oh_arr, idx16, slot_of_edge, spill = _plan_core(s_loc, d, pos)
        hperm = np.zeros((2 * NB, D), dtype=ml_dtypes.bfloat16)
        hperm[:(len(uniq) + 1) // 2] = hb[uniq[0::2]]
        hperm[NB:NB + len(uniq) // 2] = hb[uniq[1::2]]
        slab = np.zeros((SLAB_ROWS, D), dtype=ml_dtypes.float8_e4m3)
        slab[:SLICE] = h32[c * SLICE:(c + 1) * SLICE].astype(
            ml_dtypes.float8_e4m3)
        in_maps.append({
            "hperm": hperm,
            "slab": np.ascontiguousarray(slab),
            "oh": np.ascontiguousarray(oh_arr),
            "idx": np.ascontiguousarray(idx16),
        })
        plans.append((pos, slot_of_edge, spill))
    return in_maps, plans


def assemble_output(results, plans, h, src, dst):
    h32 = np.asarray(h, dtype=np.float32)
    src64 = np.asarray(src, dtype=np.int64)
    dst64 = np.asarray(dst, dtype=np.int64)
    score = np.empty(E_TOTAL, np.float64)
    for (pos, slot_of_edge, spill), r in zip(plans, results):
        scm = r["sc"].astype(np.float64)  # [P, TOT_SLOTS//P]
        ok = slot_of_edge >= 0
        sl = slot_of_edge[ok]
        score[pos[ok]] = scm[sl % P, sl // P]
        if spill:
            pp = pos[np.asarray(spill, np.int64)]
            score[pp] = np.einsum(
                "ij,ij->i", h32[src64[pp]], h32[dst64[pp]],
                dtype=np.float64)
    gmin = score.min()
    return (score != gmin).astype(np.float32).reshape(E_TOTAL, 1)


def kernel(h, src, dst):
    if "nc" not in _CACHE:
        _CACHE["nc"] = build_nc()
    nc = _CACHE["nc"]
    in_maps, plans = make_in_maps(h, src, dst)
    res = run_bass_kernel_spmd(nc, in_maps, list(range(N_CORES)))
    return assemble_output(res.results, plans, h, src, dst)


# revision 5
# speedup vs baseline: 1.0199x; 1.0199x over previous
"""DotProductPredictor kernel for trn2 (8 NeuronCores, SPMD).

score[e] = <h[src[e]], h[dst[e]]> over 600k edges vs a 100k x 128 table;
output is (score != global_min(score)) as float32 [600000, 1] — what the
reference's min-max normalize + (norm==0 ? 0 : 1) threshold produces.

Design (~202us HW; row-gather baseline was 479us). The Q7 SWDGE
descriptor-generation rate (~8.7ns/desc per queue pair, 4 pairs max) is the
hard bottleneck, so the layout minimizes gather descriptors: edges are
sharded to cores by src slice (core c owns src in [c*12500, (c+1)*12500)),
making the src side descriptor-free:

  - SRC: the slab h[c*12500:+12500] (fp8) is DMA'd to SBUF once. Edges are
    sorted by (dst_bank, src_window); segment capacity 384 = 3*128 aligns
    segments with 128-edge PE blocks, so each block needs exactly ONE fp8
    matmul (host-built one-hot stationary x slab window moving) -> PSUM
    [128, 512] holding 4 blocks of src rows [edge, feat].
  - DST: gathered bf16 via dma_gather from a per-core packed unique-dst
    table hperm (rank>>1 as int16 within two 32768-row banks selected by
    rank&1): 75264 descriptors/core in 74 gather instructions (supertiles
    of 6x1024 idx; the 1024-idx ucode cap is a hard device limit).
  - Act engine copies PSUM f32 -> SBUF bf16 so the DVE multiply runs at
    2 elem/cycle; DVE strided free-dim reduce -> scores [128, 588] -> DRAM.
  - Bank 0 starts / bank 1 ends with the short 768-slot piece to shorten
    pipeline fill/drain.
  - Host assembles: scores at [slot%128, slot//128], ~12k segment-overflow
    edges recomputed exactly (vectorized), global min + (score != min)
    threshold on host — fp8/bf16 score noise (~0.3 max) is far below the
    min gap (2.67), so the argmin is preserved exactly.
"""

import os

import numpy as np

from concourse import bass, mybir, tile
from concourse.bass_utils import run_bass_kernel_spmd
from concourse import library_config

P = 128
D = 128
N_NODES = 100000
E_TOTAL = 600000
N_CORES = 8
SLICE = N_NODES // N_CORES          # 12500 src rows per core
N_WIN = 98                          # src windows of 128 rows (last = 84)
SLAB_ROWS = N_WIN * P               # 12544 (padded with zeros)
NB = 32768                          # rows per dst bank in hperm
CAP_MAIN = 384                      # segment capacity = 3*128: block-aligned
SEGS_PER_BANK = N_WIN
FULL_PIECE_SEGS = 4                 # 4 x 384 = 1536 cols
FULL_PIECE = 4 * CAP_MAIN           # 1536 = 12*128
SHORT_PIECE = 2 * CAP_MAIN          # 768 (windows 96, 97)
N_FULL_PIECES = 24                  # windows 0..95
SUPER_PIECES = 4                    # dst supertile = 4 full pieces = 6*1024
BANK_SLOTS = N_FULL_PIECES * FULL_PIECE + SHORT_PIECE   # 37632
TOT_SLOTS = 2 * BANK_SLOTS          # 75264
RED_BLK = 512                       # reduce-matmul max N (one PSUM bank f32)

_CACHE = {}


def _seg_caps():
    return [CAP_MAIN] * 98


def _pieces():
    """Per bank: list of (piece_cols, [(window, cap), ...])."""
    caps = _seg_caps()
    pieces = []
    for i in range(N_FULL_PIECES):
        ws = list(range(4 * i, 4 * i + 4))
        pieces.append((FULL_PIECE, [(w, caps[w]) for w in ws]))
    pieces.append((SHORT_PIECE, [(96, caps[96]), (97, caps[97])]))
    return pieces


def _groups(bank):
    """Per bank: dst supertile groups of (cols, gather_sizes, [piece idx]).

    The short piece leads bank 0 (short pipeline fill) and trails bank 1
    (short drain)."""
    fulls = []
    for g in range(N_FULL_PIECES // SUPER_PIECES):
        fulls.append((SUPER_PIECES * FULL_PIECE, [1024] * 6,
                      list(range(g * SUPER_PIECES, (g + 1) * SUPER_PIECES))))
    short = (SHORT_PIECE, [768], [N_FULL_PIECES])
    return [short] + fulls if bank == 0 else fulls + [short]


IDX_COLS_TOTAL = 2 * sum(
    sum(n // 16 for n in gs) for _, gs, _ in _groups(0)
)


def _split_multi_waits(nc):
    n = 0
    for b in nc.m.functions[0].blocks:
        new_list = []
        for ins in b.instructions:
            si = ins.sync_info
            if (
                si is not None
                and si.on_wait
                and len(si.on_wait) > 1
                and not isinstance(ins, mybir.InstEventSemaphore)
            ):
                waits = list(si.on_wait)
                for w in waits[:-1]:
                    n += 1
                    ev = mybir.InstEventSemaphore(
                        name=f"wait_split_{n}",
                        opcode="EventSemaphore",
                        engine=ins.engine,
                        ins=[],
                        outs=[],
                        sync_info=mybir.SyncInfo(on_wait=[w], on_update=[]),
                    )
                    nc.inst_map[ev.name] = ev
                    new_list.append(ev)
                si.on_wait = [waits[-1]]
            new_list.append(ins)
        b.instructions[:] = new_list


def build_nc():
    nc = bass.Bass(
        num_devices=N_CORES,
        num_swdge_queues=4,
        dynamic_dma_scratch_size=16384,
    )
    bf16 = mybir.dt.bfloat16
    fp8 = mybir.dt.float8e4
    hperm = nc.dram_tensor("hperm", [2 * NB, D], bf16, kind="ExternalInput")
    slab = nc.dram_tensor("slab", [SLAB_ROWS, D], fp8, kind="ExternalInput")
    oh = nc.dram_tensor("oh", [P, TOT_SLOTS], fp8, kind="ExternalInput")
    idx = nc.dram_tensor("idx", [P, IDX_COLS_TOTAL], mybir.dt.int16,
                         kind="ExternalInput")
    sc = nc.dram_tensor("sc", [P, TOT_SLOTS // P], mybir.dt.float32,
                        kind="ExternalOutput")

    pieces = _pieces()
    with tile.TileContext(nc) as tc:
        with (
            tc.tile_pool(name="io", bufs=1) as io_pool,
            tc.tile_pool(name="dst", bufs=3) as dst_pool,
            tc.tile_pool(name="ohp", bufs=4) as oh_pool,
            tc.tile_pool(name="prd", bufs=4) as prd_pool,
            tc.tile_pool(name="srb", bufs=6) as srb_pool,
            tc.psum_pool(name="psa", bufs=6) as psa_pool,
        ):
            nc.gpsimd.load_library(library_config.mlp)
            regs = {n: nc.gpsimd.to_reg(n) for n in (1024, 768)}
            idx_sb = io_pool.tile([P, IDX_COLS_TOTAL], mybir.dt.int16)
            nc.sync.dma_start(out=idx_sb[:], in_=idx[:])
            slab_sb = io_pool.tile([P, N_WIN, D], fp8)
            nc.sync.dma_start(
                out=slab_sb[:],
                in_=slab[:].rearrange("(w p) f -> p w f", p=P),
            )
            scores_sb = io_pool.tile([P, TOT_SLOTS // P], mybir.dt.float32)

            qn = 0
            icol = 0
            slot_off = 0
            blk_off = 0
            for b in range(2):
                for gcols, gsizes, pidx in _groups(b):
                    dstS = dst_pool.tile([P, gcols], bf16, tag="dst")
                    goff = 0
                    for n in gsizes:
                        nc.gpsimd.dma_gather(
                            out_ap=dstS[:, goff:goff + n].rearrange(
                                "p (b e) -> p b e", e=D),
                            in_ap=hperm[b * NB:(b + 1) * NB, :],
                            idxs_ap=idx_sb[:, icol:icol + n // 16],
                            num_idxs=n,
                            num_idxs_reg=regs[n],
                            elem_size=D,
                            queue_num=qn % 4,
                        )
                        qn += 1
                        icol += n // 16
                        goff += n
                    poff = 0
                    for pi in pidx:
                        pc, segs = pieces[pi]
                        nb = pc // P
                        oh_t = oh_pool.tile([P, pc], fp8, tag="oh")
                        nc.sync.dma_start(
                            out=oh_t[:], in_=oh[:, slot_off:slot_off + pc])
                        prod = prd_pool.tile([P, pc], bf16, tag="prd")
                        for t0 in range(0, nb, 4):
                            tn = min(4, nb - t0)
                            pa = psa_pool.tile([P, 512], mybir.dt.float32,
                                               tag="psa")
                            for bi in range(t0, t0 + tn):
                                w = segs[bi * P // CAP_MAIN][0]
                                nc.tensor.matmul(
                                    out=pa[:, (bi - t0) * P:
                                           (bi - t0 + 1) * P],
                                    lhsT=oh_t[:, bi * P:(bi + 1) * P],
                                    rhs=slab_sb[:, w, :],
                                    start=True,
                                    stop=True,
                                )
                            srcb = srb_pool.tile([P, 512], bf16, tag="srb")
                            nc.scalar.copy(out=srcb[:, :tn * P],
                                           in_=pa[:, :tn * P])
                            nc.vector.tensor_tensor(
                                out=prod[:, t0 * P:(t0 + tn) * P],
                                in0=srcb[:, :tn * P],
                                in1=dstS[:, poff + t0 * P:
                                         poff + (t0 + tn) * P],
                                op=mybir.AluOpType.mult,
                            )
                        nc.vector.tensor_reduce(
                            out=scores_sb[:, blk_off:blk_off + nb],
                            in_=prod[:].rearrange("p (b f) -> p b f", f=D),
                            axis=mybir.AxisListType.X,
                            op=mybir.AluOpType.add,
                        )
                        slot_off += pc
                        blk_off += nb
                        poff += pc
            nc.sync.dma_start(out=sc[:], in_=scores_sb[:])

    _split_multi_waits(nc)
    mybir.codegen_inst_isa_subclasses(nc)
    return nc


def _plan_core(s_loc, d, positions):
    """Host plan for one core.

    s_loc: local src (0..12499), d: global dst, positions: original edge idx.
    Returns in_map tensors (oh bf16, idx int16 wrapped, hperm-fill info) and
    (slot_of_edge over this core's edges, spill list of local edge indices).
    """
    import ml_dtypes

    n = len(s_loc)
    uniq, rank = np.unique(d, return_inverse=True)
    nu = len(uniq)
    assert nu <= 2 * NB, nu
    lidx = (rank >> 1).astype(np.int16)
    bank = (rank & 1).astype(np.int8)
    win = (s_loc // P).astype(np.int32)

    caps = _seg_caps()
    oh_k = np.zeros(TOT_SLOTS, np.int64)       # one-hot row per slot
    oh_on = np.zeros(TOT_SLOTS, bool)
    idx_vals = np.zeros(TOT_SLOTS, np.int16)   # gather idx per slot
    slot_of_edge = np.full(n, -1, np.int64)
    spill = []

    # segment slot offsets in the fixed (per-bank group-order) layout
    pieces = _pieces()
    seg_off = {}
    off = 0
    for b in range(2):
        for gcols, gsizes, pidx in _groups(b):
            for pi in pidx:
                for w, cap in pieces[pi][1]:
                    seg_off[(b, w)] = off
                    off += cap
    assert off == TOT_SLOTS

    order = np.lexsort((s_loc, win, bank))
    key = bank[order].astype(np.int64) * 1000 + win[order]
    bounds = np.flatnonzero(np.r_[True, np.diff(key) != 0])
    bounds = np.r_[bounds, len(order)]
    for gi in range(len(bounds) - 1):
        members = order[bounds[gi]:bounds[gi + 1]]
        b = int(bank[members[0]])
        w = int(win[members[0]])
        cap = caps[w]
        if len(members) > cap:
            spill.extend(members[cap:].tolist())
            members = members[:cap]
        base = seg_off[(b, w)]
        k = len(members)
        slot_of_edge[members] = base + np.arange(k)
        sl = base + np.arange(cap)
        oh_k[sl[:k]] = s_loc[members] - w * P
        oh_on[sl[:k]] = True
        idx_vals[sl[:k]] = lidx[members]
        if k:
            oh_k[sl[k:]] = s_loc[members[-1]] - w * P
            oh_on[sl[k:]] = True
            idx_vals[sl[k:]] = lidx[members[-1]]
        # empty segment: oh stays zero -> score 0, excluded from edge map

    oh_arr = np.zeros((P, TOT_SLOTS), dtype=ml_dtypes.float8_e4m3)
    on = np.flatnonzero(oh_on)
    oh_arr[oh_k[on], on] = 1.0

    # idx16 wrapped per gather, in program order (bank-contiguous chunks)
    idx16 = np.zeros((16, IDX_COLS_TOTAL), np.int16)
    icol = 0
    slot = 0
    for b in range(2):
        for gcols, gsizes, _ in _groups(b):
            goff = 0
            for gsz in gsizes:
                vals = idx_vals[slot + goff: slot + goff + gsz]
                idx16[:, icol:icol + gsz // 16] = vals.reshape(
                    gsz // 16, 16).T
                icol += gsz // 16
                goff += gsz
            slot += gcols
    idx16_full = np.tile(idx16, (8, 1))
    return uniq, oh_arr, idx16_full, slot_of_edge, spill


def make_in_maps(h, src, dst):
    import ml_dtypes

    h32 = np.asarray(h, dtype=np.float32)
    hb = h32.astype(ml_dtypes.bfloat16)
    src64 = np.asarray(src, dtype=np.int64)
    dst64 = np.asarray(dst, dtype=np.int64)
    owner = src64 // SLICE
    in_maps, plans = [], []
    for c in range(N_CORES):
        pos = np.flatnonzero(owner == c)
        s_loc = (src64[pos] - c * SLICE).astype(np.int64)
        d = dst64[pos]
        uniq, oh_arr, idx16, slot_of_edge, spill = _plan_core(s_loc, d, pos)
        hperm = np.zeros((2 * NB, D), dtype=ml_dtypes.bfloat16)
        hperm[:(len(uniq) + 1) // 2] = hb[uniq[0::2]]
        hperm[NB:NB + len(uniq) // 2] = hb[uniq[1::2]]
        slab = np.zeros((SLAB_ROWS, D), dtype=ml_dtypes.float8_e4m3)
        slab[:SLICE] = h32[c * SLICE:(c + 1) * SLICE].astype(
            ml_dtypes.float8_e4m3)
        in_maps.append({
            "hperm": hperm,
            "slab": np.ascontiguousarray(slab),
            "oh": np.ascontiguousarray(oh_arr),
            "idx": np.ascontiguousarray(idx16),
        })
        plans.append((pos, slot_of_edge, spill))
    return in_maps, plans


def assemble_output(results, plans, h, src, dst):
    h32 = np.asarray(h, dtype=np.float32)
    src64 = np.asarray(src, dtype=np.int64)
    dst64 = np.asarray(dst, dtype=np.int64)
    score = np.empty(E_TOTAL, np.float64)
    for (pos, slot_of_edge, spill), r in zip(plans, results):
        scm = r["sc"].astype(np.float64)  # [P, TOT_SLOTS//P]
        ok = slot_of_edge >= 0
        sl = slot_of_edge[ok]
        score[pos[ok]] = scm[sl % P, sl // P]
        if spill:
            pp = pos[np.asarray(spill, np.int64)]
            score[pp] = np.einsum(
                "ij,ij->i", h32[src64[pp]], h32[dst64[pp]],
                dtype=np.float64)
    gmin = score.min()
    return (score != gmin).astype(np.float32).reshape(E_TOTAL, 1)


def kernel(h, src, dst):
    if "nc" not in _CACHE:
        _CACHE["nc"] = build_nc()
    nc = _CACHE["nc"]
    in_maps, plans = make_in_maps(h, src, dst)
    res = run_bass_kernel_spmd(nc, in_maps, list(range(N_CORES)))
    return assemble_output(res.results, plans, h, src, dst)


# revision 6
# speedup vs baseline: 1.0306x; 1.0104x over previous
"""DotProductPredictor kernel for trn2 (8 NeuronCores, SPMD).

score[e] = <h[src[e]], h[dst[e]]> over 600k edges vs a 100k x 128 table;
output is (score != global_min(score)) as float32 [600000, 1] — what the
reference's min-max normalize + (norm==0 ? 0 : 1) threshold produces.

Design (~202us HW; row-gather baseline was 479us). The Q7 SWDGE
descriptor-generation rate (~8.7ns/desc per queue pair, 4 pairs max) is the
hard bottleneck, so the layout minimizes gather descriptors: edges are
sharded to cores by src slice (core c owns src in [c*12500, (c+1)*12500)),
making the src side descriptor-free:

  - SRC: the slab h[c*12500:+12500] (fp8) is DMA'd to SBUF once. Edges are
    sorted by (dst_bank, src_window); segment capacity 384 = 3*128 aligns
    segments with 128-edge PE blocks, so each block needs exactly ONE fp8
    matmul (host-built one-hot stationary x slab window moving) -> PSUM
    [128, 512] holding 4 blocks of src rows [edge, feat].
  - DST: gathered bf16 via dma_gather from a per-core packed unique-dst
    table hperm (rank>>1 as int16 within two 32768-row banks selected by
    rank&1): 75264 descriptors/core in 50 gather instructions (supertiles of
    4x1536 idx with single_packet=False — the apparent 1024-idx cap is the
    64-descriptor single-packet ceiling, not an index limit).
  - Act engine copies PSUM f32 -> SBUF bf16 so the DVE multiply runs at
    2 elem/cycle; DVE strided free-dim reduce -> scores [128, 588] -> DRAM.
  - Bank 0 starts / bank 1 ends with the short 768-slot piece to shorten
    pipeline fill/drain.
  - Host assembles: scores at [slot%128, slot//128], ~12k segment-overflow
    edges recomputed exactly (vectorized), global min + (score != min)
    threshold on host — fp8/bf16 score noise (~0.3 max) is far below the
    min gap (2.67), so the argmin is preserved exactly.
"""

import os

import numpy as np

from concourse import bass, mybir, tile
from concourse.bass_utils import run_bass_kernel_spmd
from concourse import library_config

P = 128
D = 128
N_NODES = 100000
E_TOTAL = 600000
N_CORES = 8
SLICE = N_NODES // N_CORES          # 12500 src rows per core
N_WIN = 98                          # src windows of 128 rows (last = 84)
SLAB_ROWS = N_WIN * P               # 12544 (padded with zeros)
NB = 32768                          # rows per dst bank in hperm
CAP_MAIN = 384                      # segment capacity = 3*128: block-aligned
SEGS_PER_BANK = N_WIN
FULL_PIECE_SEGS = 4                 # 4 x 384 = 1536 cols
FULL_PIECE = 4 * CAP_MAIN           # 1536 = 12*128
SHORT_PIECE = 2 * CAP_MAIN          # 768 (windows 96, 97)
N_FULL_PIECES = 24                  # windows 0..95
SUPER_PIECES = 4                    # dst supertile = 4 full pieces = 6*1024
BANK_SLOTS = N_FULL_PIECES * FULL_PIECE + SHORT_PIECE   # 37632
TOT_SLOTS = 2 * BANK_SLOTS          # 75264
RED_BLK = 512                       # reduce-matmul max N (one PSUM bank f32)

_CACHE = {}


def _seg_caps():
    return [CAP_MAIN] * 98


def _pieces():
    """Per bank: list of (piece_cols, [(window, cap), ...])."""
    caps = _seg_caps()
    pieces = []
    for i in range(N_FULL_PIECES):
        ws = list(range(4 * i, 4 * i + 4))
        pieces.append((FULL_PIECE, [(w, caps[w]) for w in ws]))
    pieces.append((SHORT_PIECE, [(96, caps[96]), (97, caps[97])]))
    return pieces


def _groups(bank):
    """Per bank: dst supertile groups of (cols, gather_sizes, [piece idx]).

    The short piece leads bank 0 (short pipeline fill) and trails bank 1
    (short drain)."""
    fulls = []
    for g in range(N_FULL_PIECES // SUPER_PIECES):
        fulls.append((SUPER_PIECES * FULL_PIECE, [1536] * 4,
                      list(range(g * SUPER_PIECES, (g + 1) * SUPER_PIECES))))
    short = (SHORT_PIECE, [768], [N_FULL_PIECES])
    return [short] + fulls if bank == 0 else fulls + [short]


IDX_COLS_TOTAL = 2 * sum(
    sum(n // 16 for n in gs) for _, gs, _ in _groups(0)
)


def _split_multi_waits(nc):
    n = 0
    for b in nc.m.functions[0].blocks:
        new_list = []
        for ins in b.instructions:
            si = ins.sync_info
            if (
                si is not None
                and si.on_wait
                and len(si.on_wait) > 1
                and not isinstance(ins, mybir.InstEventSemaphore)
            ):
                waits = list(si.on_wait)
                for w in waits[:-1]:
                    n += 1
                    ev = mybir.InstEventSemaphore(
                        name=f"wait_split_{n}",
                        opcode="EventSemaphore",
                        engine=ins.engine,
                        ins=[],
                        outs=[],
                        sync_info=mybir.SyncInfo(on_wait=[w], on_update=[]),
                    )
                    nc.inst_map[ev.name] = ev
                    new_list.append(ev)
                si.on_wait = [waits[-1]]
            new_list.append(ins)
        b.instructions[:] = new_list


def build_nc():
    nc = bass.Bass(
        num_devices=N_CORES,
        num_swdge_queues=4,
        dynamic_dma_scratch_size=16384,
    )
    bf16 = mybir.dt.bfloat16
    fp8 = mybir.dt.float8e4
    hperm = nc.dram_tensor("hperm", [2 * NB, D], bf16, kind="ExternalInput")
    slab = nc.dram_tensor("slab", [SLAB_ROWS, D], fp8, kind="ExternalInput")
    oh = nc.dram_tensor("oh", [P, TOT_SLOTS], fp8, kind="ExternalInput")
    idx = nc.dram_tensor("idx", [P, IDX_COLS_TOTAL], mybir.dt.int16,
                         kind="ExternalInput")
    sc = nc.dram_tensor("sc", [P, TOT_SLOTS // P], mybir.dt.float32,
                        kind="ExternalOutput")

    pieces = _pieces()
    with tile.TileContext(nc) as tc:
        with (
            tc.tile_pool(name="io", bufs=1) as io_pool,
            tc.tile_pool(name="dst", bufs=3) as dst_pool,
            tc.tile_pool(name="ohp", bufs=4) as oh_pool,
            tc.tile_pool(name="prd", bufs=4) as prd_pool,
            tc.tile_pool(name="srb", bufs=6) as srb_pool,
            tc.psum_pool(name="psa", bufs=6) as psa_pool,
        ):
            nc.gpsimd.load_library(library_config.mlp)
            regs = {n: nc.gpsimd.to_reg(n) for n in (1536, 768, 16)}
            # warmup on q0: pays the ~6us Q7 IRAM load while inputs stream;
            # real gathers start on q1 so the first one is not behind it
            wi = io_pool.tile([P, 1], mybir.dt.int16)
            nc.vector.memset(wi[:], 0)
            wg = io_pool.tile([P, 1, D], mybir.dt.bfloat16)
            nc.gpsimd.dma_gather(
                out_ap=wg[:],
                in_ap=hperm[0:NB, :],
                idxs_ap=wi[:],
                num_idxs=16,
                num_idxs_reg=regs[16],
                elem_size=D,
                queue_num=0,
            )
            # the first (short, 48-col) gather's indices load separately so
            # it does not wait for the full 1.2MB idx DMA
            idx_sb0 = io_pool.tile([P, 48], mybir.dt.int16)
            nc.sync.dma_start(out=idx_sb0[:], in_=idx[:, 0:48])
            idx_sb = io_pool.tile([P, IDX_COLS_TOTAL], mybir.dt.int16)
            nc.sync.dma_start(out=idx_sb[:], in_=idx[:])
            slab_sb = io_pool.tile([P, N_WIN, D], fp8)
            nc.sync.dma_start(
                out=slab_sb[:],
                in_=slab[:].rearrange("(w p) f -> p w f", p=P),
            )
            scores_sb = io_pool.tile([P, TOT_SLOTS // P], mybir.dt.float32)

            qn = 1
            icol = 0
            slot_off = 0
            blk_off = 0
            for b in range(2):
                for gcols, gsizes, pidx in _groups(b):
                    dstS = dst_pool.tile([P, gcols], bf16, tag="dst")
                    goff = 0
                    for n in gsizes:
                        first = icol == 0
                        isrc = idx_sb0 if first else idx_sb
                        nc.gpsimd.dma_gather(
                            out_ap=dstS[:, goff:goff + n].rearrange(
                                "p (b e) -> p b e", e=D),
                            in_ap=hperm[b * NB:(b + 1) * NB, :],
                            idxs_ap=isrc[:, icol:icol + n // 16],
                            num_idxs=n,
                            num_idxs_reg=regs[n],
                            elem_size=D,
                            single_packet=(n <= 1024),
                            queue_num=qn % 4,
                        )
                        qn += 1
                        icol += n // 16
                        goff += n
                    poff = 0
                    for pi in pidx:
                        pc, segs = pieces[pi]
                        nb = pc // P
                        oh_t = oh_pool.tile([P, pc], fp8, tag="oh")
                        nc.sync.dma_start(
                            out=oh_t[:], in_=oh[:, slot_off:slot_off + pc])
                        prod = prd_pool.tile([P, pc], bf16, tag="prd")
                        for t0 in range(0, nb, 4):
                            tn = min(4, nb - t0)
                            pa = psa_pool.tile([P, 512], mybir.dt.float32,
                                               tag="psa")
                            for bi in range(t0, t0 + tn):
                                w = segs[bi * P // CAP_MAIN][0]
                                nc.tensor.matmul(
                                    out=pa[:, (bi - t0) * P:
                                           (bi - t0 + 1) * P],
                                    lhsT=oh_t[:, bi * P:(bi + 1) * P],
                                    rhs=slab_sb[:, w, :],
                                    start=True,
                                    stop=True,
                                )
                            srcb = srb_pool.tile([P, 512], bf16, tag="srb")
                            nc.scalar.copy(out=srcb[:, :tn * P],
                                           in_=pa[:, :tn * P])
                            nc.vector.tensor_tensor(
                                out=prod[:, t0 * P:(t0 + tn) * P],
                                in0=srcb[:, :tn * P],
                                in1=dstS[:, poff + t0 * P:
                                         poff + (t0 + tn) * P],
                                op=mybir.AluOpType.mult,
                            )
                        nc.vector.tensor_reduce(
                            out=scores_sb[:, blk_off:blk_off + nb],
                            in_=prod[:].rearrange("p (b f) -> p b f", f=D),
                            axis=mybir.AxisListType.X,
                            op=mybir.AluOpType.add,
                        )
                        slot_off += pc
                        blk_off += nb
                        poff += pc
            nc.sync.dma_start(out=sc[:], in_=scores_sb[:])

    _split_multi_waits(nc)
    mybir.codegen_inst_isa_subclasses(nc)
    return nc


def _plan_core(s_loc, d, positions):
    """Host plan for one core.

    s_loc: local src (0..12499), d: global dst, positions: original edge idx.
    Returns in_map tensors (oh bf16, idx int16 wrapped, hperm-fill info) and
    (slot_of_edge over this core's edges, spill list of local edge indices).
    """
    import ml_dtypes

    n = len(s_loc)
    uniq, rank = np.unique(d, return_inverse=True)
    nu = len(uniq)
    assert nu <= 2 * NB, nu
    lidx = (rank >> 1).astype(np.int16)
    bank = (rank & 1).astype(np.int8)
    win = (s_loc // P).astype(np.int32)

    caps = _seg_caps()
    oh_k = np.zeros(TOT_SLOTS, np.int64)       # one-hot row per slot
    oh_on = np.zeros(TOT_SLOTS, bool)
    idx_vals = np.zeros(TOT_SLOTS, np.int16)   # gather idx per slot
    slot_of_edge = np.full(n, -1, np.int64)
    spill = []

    # segment slot offsets in the fixed (per-bank group-order) layout
    pieces = _pieces()
    seg_off = {}
    off = 0
    for b in range(2):
        for gcols, gsizes, pidx in _groups(b):
            for pi in pidx:
                for w, cap in pieces[pi][1]:
                    seg_off[(b, w)] = off
                    off += cap
    assert off == TOT_SLOTS

    order = np.lexsort((s_loc, win, bank))
    key = bank[order].astype(np.int64) * 1000 + win[order]
    bounds = np.flatnonzero(np.r_[True, np.diff(key) != 0])
    bounds = np.r_[bounds, len(order)]
    for gi in range(len(bounds) - 1):
        members = order[bounds[gi]:bounds[gi + 1]]
        b = int(bank[members[0]])
        w = int(win[members[0]])
        cap = caps[w]
        if len(members) > cap:
            spill.extend(members[cap:].tolist())
            members = members[:cap]
        base = seg_off[(b, w)]
        k = len(members)
        slot_of_edge[members] = base + np.arange(k)
        sl = base + np.arange(cap)
        oh_k[sl[:k]] = s_loc[members] - w * P
        oh_on[sl[:k]] = True
        idx_vals[sl[:k]] = lidx[members]
        if k:
            oh_k[sl[k:]] = s_loc[members[-1]] - w * P
            oh_on[sl[k:]] = True
            idx_vals[sl[k:]] = lidx[members[-1]]
        # empty segment: oh stays zero -> score 0, excluded from edge map

    oh_arr = np.zeros((P, TOT_SLOTS), dtype=ml_dtypes.float8_e4m3)
    on = np.flatnonzero(oh_on)
    oh_arr[oh_k[on], on] = 1.0

    # idx16 wrapped per gather, in program order (bank-contiguous chunks)
    idx16 = np.zeros((16, IDX_COLS_TOTAL), np.int16)
    icol = 0
    slot = 0
    for b in range(2):
        for gcols, gsizes, _ in _groups(b):
            goff = 0
            for gsz in gsizes:
                vals = idx_vals[slot + goff: slot + goff + gsz]
                idx16[:, icol:icol + gsz // 16] = vals.reshape(
                    gsz // 16, 16).T
                icol += gsz // 16
                goff += gsz
            slot += gcols
    idx16_full = np.tile(idx16, (8, 1))
    return uniq, oh_arr, idx16_full, slot_of_edge, spill


def make_in_maps(h, src, dst):
    import ml_dtypes

    h32 = np.asarray(h, dtype=np.float32)
    hb = h32.astype(ml_dtypes.bfloat16)
    src64 = np.asarray(src, dtype=np.int64)
    dst64 = np.asarray(dst, dtype=np.int64)
    owner = src64 // SLICE
    in_maps, plans = [], []
    for c in range(N_CORES):
        pos = np.flatnonzero(owner == c)
        s_loc = (src64[pos] - c * SLICE).astype(np.int64)
        d = dst64[pos]
        uniq, oh_arr, idx16, slot_of_edge, spill = _plan_core(s_loc, d, pos)
        hperm = np.zeros((2 * NB, D), dtype=ml_dtypes.bfloat16)
        hperm[:(len(uniq) + 1) // 2] = hb[uniq[0::2]]
        hperm[NB:NB + len(uniq) // 2] = hb[uniq[1::2]]
        slab = np.zeros((SLAB_ROWS, D), dtype=ml_dtypes.float8_e4m3)
        slab[:SLICE] = h32[c * SLICE:(c + 1) * SLICE].astype(
            ml_dtypes.float8_e4m3)
        in_maps.append({
            "hperm": hperm,
            "slab": np.ascontiguousarray(slab),
            "oh": np.ascontiguousarray(oh_arr),
            "idx": np.ascontiguousarray(idx16),
        })
        plans.append((pos, slot_of_edge, spill))
    return in_maps, plans


def assemble_output(results, plans, h, src, dst):
    h32 = np.asarray(h, dtype=np.float32)
    src64 = np.asarray(src, dtype=np.int64)
    dst64 = np.asarray(dst, dtype=np.int64)
    score = np.empty(E_TOTAL, np.float64)
    for (pos, slot_of_edge, spill), r in zip(plans, results):
        scm = r["sc"].astype(np.float64)  # [P, TOT_SLOTS//P]
        ok = slot_of_edge >= 0
        sl = slot_of_edge[ok]
        score[pos[ok]] = scm[sl % P, sl // P]
        if spill:
            pp = pos[np.asarray(spill, np.int64)]
            score[pp] = np.einsum(
                "ij,ij->i", h32[src64[pp]], h32[dst64[pp]],
                dtype=np.float64)
    gmin = score.min()
    return (score != gmin).astype(np.float32).reshape(E_TOTAL, 1)


def kernel(h, src, dst):
    if "nc" not in _CACHE:
        _CACHE["nc"] = build_nc()
    nc = _CACHE["nc"]
    in_maps, plans = make_in_maps(h, src, dst)
    res = run_bass_kernel_spmd(nc, in_maps, list(range(N_CORES)))
    return assemble_output(res.results, plans, h, src, dst)
